# revision 15
# baseline (speedup 1.0000x reference)
"""Trainium2 Bass kernel for nn_MoE_multirules (moe_routing).

Computes, for x[B,D], gating weights Wg[D,2], expert weights Wml/Wr[D,C]:
    gate = softmax(x @ Wg + bg)                        [B,2]
    y_rule = relu(x @ Wr + br) * support_mask[:,None]  [B,C]
    mask_support = sum(y_rule, -1)                     [B]
    y_ml = x @ Wml + bml                               [B,C]
    mix  = g0*y_ml + g1*y_rule        (row 0)
         = g0*(y_ml + g1*y_rule)      (rows > 0)
    y    = where(mask_support != 0, mix, y_ml)
Returns (y, gate, mask_support).

Strategy: pure data parallel over 8 NeuronCores (2048 rows each, weights
replicated). The two D*C matmuls (99.8% of FLOPs) run on the PE at fp32r
full rate (N=512 moving); x is pre-transposed host-side so the contraction
dim D sits on partitions without burning PE time on transposes, and the
tiny gating network (x @ Wg is 0.1% of FLOPs) plus the per-row softmax
coefficients are computed host-side in fp32 — measured on HW, N=2 matmuls
and ACT-table switches cost more than the whole device epilogue.

Per 128-row tile on device:
  - 8 accumulating fp32r matmuls per output half into PSUM for each expert,
  - relu on ScalarE with per-row scale b*mask folded in (relu(b*m*z) ==
    b*m*relu(z) for b*m >= 0) and accum_out producing the scaled row sum,
  - y = (psum_ml * a) + y_rule_scaled as one fused DVE op per half.
Per-row coefficients (host): b = g1 on global row 0 else g0*g1; device
derives a = g0 if any rule fired (scaled row sum != 0) else 1, and
mask_support = row_sum / b.
"""

import numpy as np

B, D, C = 16384, 1024, 1024
N_CORES = 8
BS = B // N_CORES          # 2048 rows per core
P = 128                    # partitions
TB = BS // P               # 16 row-tiles per core
KC = D // P                # 8 contraction chunks
NT = 512                   # moving free dim per matmul
CT = C // NT               # 2 column tiles

_BUILD_CACHE = {}


def _build(has_bml, has_br, repeat=1):
    import concourse.tile as tile
    from concourse import bacc, mybir
    from concourse.bass import ts

    f32 = mybir.dt.float32
    f32r = mybir.dt.float32r
    Alu = mybir.AluOpType
    Act = mybir.ActivationFunctionType

    nc = bacc.Bacc("TRN2", target_bir_lowering=False, debug=False)

    # x pre-transposed host-side: [p, k, b] = x[b, k*P + p]
    xt_d = nc.dram_tensor("xt", [P, KC, BS], f32r, kind="ExternalInput").ap()
    wml_d = nc.dram_tensor("wml", [D, C], f32r, kind="ExternalInput").ap()
    wr_d = nc.dram_tensor("wr", [D, C], f32r, kind="ExternalInput").ap()
    # host-computed per-row coefficients, [p, t] = row t*P + p
    g0_d = nc.dram_tensor("g0v", [P, TB], f32, kind="ExternalInput").ap()
    u_d = nc.dram_tensor("uv", [P, TB], f32, kind="ExternalInput").ap()     # 1-g0
    brl_d = nc.dram_tensor("brlv", [P, TB], f32, kind="ExternalInput").ap() # b*mask
    binv_d = nc.dram_tensor("binvv", [P, TB], f32, kind="ExternalInput").ap()  # 1/b
    bias_d = {}
    if has_bml:
        bias_d["bml"] = nc.dram_tensor("bml", [C], f32r, kind="ExternalInput").ap()
    if has_br:
        bias_d["br"] = nc.dram_tensor("br", [C], f32r, kind="ExternalInput").ap()

    y_d = nc.dram_tensor("y", [BS, C], f32, kind="ExternalOutput").ap()
    ms_d = nc.dram_tensor("ms", [P, TB], f32, kind="ExternalOutput").ap()

    with tile.TileContext(nc) as tc:
        with (
            tc.tile_pool(name="wpool", bufs=1) as wpool,
            tc.tile_pool(name="cpool", bufs=1) as cpool,
            tc.tile_pool(name="yrpool", bufs=4) as yrpool,
            tc.tile_pool(name="ypool", bufs=4) as ypool,
            tc.tile_pool(name="scpool", bufs=3) as scpool,
            tc.tile_pool(name="pacc", bufs=8, space="PSUM") as pacc_pool,
        ):
            bias_sb = {}
            if bias_d:
                onesf = cpool.tile([1, P], f32, tag="onesf")
                nc.vector.memset(onesf[:], 1.0)
                ones_t = cpool.tile([1, P], f32r, tag="ones")
                nc.vector.tensor_copy(ones_t[:], onesf[:])
                for key, ap in bias_d.items():
                    bt = cpool.tile([1, ap.shape[0]], f32r, tag=f"b_{key}",
                                    name=f"b_{key}")
                    nc.sync.dma_start(bt[:], ap.rearrange("(o c) -> o c", o=1))
                    bias_sb[key] = bt

            coef_sb = {}
            for key, ap in [("g0", g0_d), ("u", u_d), ("brl", brl_d),
                            ("binv", binv_d)]:
                ct_ = wpool.tile([P, TB], f32, tag=f"c_{key}", name=f"c_{key}")
                nc.sync.dma_start(ct_[:], ap)
                coef_sb[key] = ct_

            ms_sb = wpool.tile([P, TB], f32, tag="ms_acc")

            wml_r = wml_d.rearrange("(k p) c -> p k c", p=P)
            wr_r = wr_d.rearrange("(k p) c -> p k c", p=P)
            for rep in range(repeat):
                # loads sit inside the repeat loop so repeat>1 timing variants
                # account for them; with repeat=1 this is the plain kernel.
                # Fine-grained (256 KB) chunks, emitted in k order with the
                # rule weights first — tiles consume them first below.
                xt_sb = wpool.tile([P, KC, BS], f32r, tag="xt")
                wml_sb = wpool.tile([P, KC, C], f32r, tag="wml")
                wr_sb = wpool.tile([P, KC, C], f32r, tag="wr")
                for k in range(KC):
                    for h in range(2):
                        nc.sync.dma_start(
                            xt_sb[:, k, ts(h, BS // 2)], xt_d[:, k, ts(h, BS // 2)]
                        )
                        nc.sync.dma_start(
                            wr_sb[:, k, ts(h, NT)], wr_r[:, k, ts(h, NT)]
                        )
                        nc.sync.dma_start(
                            wml_sb[:, k, ts(h, NT)], wml_r[:, k, ts(h, NT)]
                        )

                for t in range(TB):
                    # ---- rule expert first: its relu -> acf chain hides
                    # under the ml matmuls that follow ----
                    ps_r = [
                        pacc_pool.tile([P, NT], f32, tag="acc", name=f"ps_r{c}")
                        for c in range(CT)
                    ]
                    for k in range(KC):
                        lhs = xt_sb[:, k, ts(t, P)]
                        for c in range(CT):
                            nc.tensor.matmul(
                                ps_r[c][:], lhs, wr_sb[:, k, ts(c, NT)],
                                start=(k == 0), stop=(k == KC - 1) and not has_br,
                            )
                    if has_br:
                        for c in range(CT):
                            nc.tensor.matmul(
                                ps_r[c][:], ones_t[:],
                                bias_sb["br"][:, ts(c, NT)],
                                start=False, stop=True,
                            )

                    # scaled relu + row-sum on ScalarE (frees the r banks)
                    brl = coef_sb["brl"][:, t : t + 1]
                    yr = []
                    rs = []
                    for c in range(CT):
                        yr_c = yrpool.tile([P, NT], f32, tag="yr", name=f"yr{c}")
                        rs_c = scpool.tile([P, 1], f32, tag=f"rs{c}", name=f"rs{c}")
                        nc.scalar.activation(
                            yr_c[:], ps_r[c][:], Act.Relu,
                            scale=brl, accum_out=rs_c[:],
                        )
                        yr.append(yr_c)
                        rs.append(rs_c)
                    rsum = scpool.tile([P, 1], f32, tag="rsum")
                    nc.vector.tensor_add(rsum[:], rs[0][:], rs[1][:])

                    # supported <=> rsum != 0 ; a = g0 + (1-g0)*[rsum == 0]
                    wu = scpool.tile([P, 1], f32, tag="wu")
                    nc.vector.tensor_scalar(wu[:], rsum[:], 0.0, None, Alu.is_equal)
                    au = scpool.tile([P, 1], f32, tag="au")
                    nc.vector.tensor_mul(au[:], wu[:], coef_sb["u"][:, t : t + 1])
                    acf = scpool.tile([P, 1], f32, tag="acf")
                    nc.vector.tensor_add(acf[:], au[:], coef_sb["g0"][:, t : t + 1])

                    # mask_support = rsum / b  (0 for unsupported rows)
                    nc.vector.tensor_mul(
                        ms_sb[:, t : t + 1], rsum[:], coef_sb["binv"][:, t : t + 1]
                    )

                    # ---- ml expert; acf is ready by the time it finishes ----
                    ps_ml = [
                        pacc_pool.tile([P, NT], f32, tag="acc", name=f"ps_ml{c}")
                        for c in range(CT)
                    ]
                    for k in range(KC):
                        lhs = xt_sb[:, k, ts(t, P)]
                        for c in range(CT):
                            nc.tensor.matmul(
                                ps_ml[c][:], lhs, wml_sb[:, k, ts(c, NT)],
                                start=(k == 0), stop=(k == KC - 1) and not has_bml,
                            )
                    if has_bml:
                        for c in range(CT):
                            nc.tensor.matmul(
                                ps_ml[c][:], ones_t[:],
                                bias_sb["bml"][:, ts(c, NT)],
                                start=False, stop=True,
                            )

                    # final mix: y = a*psum_ml + yr (frees the ml banks)
                    for c in range(CT):
                        y_sb = ypool.tile([P, NT], f32, tag="y", name=f"y{c}")
                        nc.vector.scalar_tensor_tensor(
                            y_sb[:], ps_ml[c][:], acf[:, 0:1], yr[c][:],
                            Alu.mult, Alu.add,
                        )
                        nc.sync.dma_start(y_d[ts(t, P), ts(c, NT)], y_sb[:])

            nc.sync.dma_start(ms_d[:], ms_sb[:])

    nc.compile()
    return nc


def _prepare(x, Wg, bg, Wml, bml, Wr, br, support_mask):
    """Host-side prep: flags, per-core in_maps, and the gate output."""
    x = np.ascontiguousarray(np.asarray(x, dtype=np.float32))
    Wg = np.asarray(Wg, dtype=np.float32)
    Wml = np.ascontiguousarray(np.asarray(Wml, dtype=np.float32))
    Wr = np.ascontiguousarray(np.asarray(Wr, dtype=np.float32))
    bg = np.asarray(bg, dtype=np.float32)
    bml = np.asarray(bml, dtype=np.float32)
    br = np.asarray(br, dtype=np.float32)
    support_mask = np.asarray(support_mask)

    flags = (bool(bml.any()), bool(br.any()))

    # ---- host-side gating network (0.1% of FLOPs) + per-row coefficients ----
    logits = x @ Wg + bg                       # [B, 2] fp32
    m = logits.max(axis=1, keepdims=True)
    e = np.exp(logits - m)
    gate = (e / e.sum(axis=1, keepdims=True)).astype(np.float32)
    g0 = np.ascontiguousarray(gate[:, 0])
    g1 = np.ascontiguousarray(gate[:, 1])
    u = (np.float32(1.0) - g0).astype(np.float32)
    ssel = g0.copy()
    ssel[0] = np.float32(1.0)                  # row-0 asymmetry: b = g1 there
    b_coef = (g1 * ssel).astype(np.float32)
    mask_f = support_mask.astype(np.float32)
    brl = (b_coef * mask_f).astype(np.float32)
    binv = (np.float32(1.0) / b_coef).astype(np.float32)

    def per_core_cols(v, sl):                  # [BS] -> [P, TB]
        return np.ascontiguousarray(v[sl].reshape(TB, P).T)

    in_maps = []
    for cix in range(N_CORES):
        sl = slice(cix * BS, (cix + 1) * BS)
        xs = x[sl]
        # [p, k, b] = xs[b, k*P + p]
        xt = np.ascontiguousarray(xs.T.reshape(KC, P, BS).transpose(1, 0, 2))
        m_ = {
            "xt": xt,
            "wml": Wml,
            "wr": Wr,
            "g0v": per_core_cols(g0, sl),
            "uv": per_core_cols(u, sl),
            "brlv": per_core_cols(brl, sl),
            "binvv": per_core_cols(binv, sl),
        }
        if flags[0]:
            m_["bml"] = bml
        if flags[1]:
            m_["br"] = br
        in_maps.append(m_)
    return flags, in_maps, gate


def kernel(x, Wg, bg, Wml, bml, Wr, br, support_mask):
    from concourse.bass_utils import run_bass_kernel_spmd

    flags, in_maps, gate = _prepare(x, Wg, bg, Wml, bml, Wr, br, support_mask)
    if flags not in _BUILD_CACHE:
        _BUILD_CACHE[flags] = _build(*flags)
    nc = _BUILD_CACHE[flags]

    res = run_bass_kernel_spmd(nc, in_maps, core_ids=list(range(N_CORES)))

    y = np.concatenate([res.results[c]["y"] for c in range(N_CORES)], axis=0)
    ms = np.concatenate(
        [res.results[c]["ms"].T.reshape(BS) for c in range(N_CORES)], axis=0
    )
    return y, gate, ms


# revision 16
# speedup vs baseline: 1.6468x; 1.6468x over previous
"""Trainium2 Bass kernel for nn_MoE_multirules (moe_routing).

Computes, for x[B,D], gating weights Wg[D,2], expert weights Wml/Wr[D,C]:
    gate = softmax(x @ Wg + bg)                        [B,2]
    y_rule = relu(x @ Wr + br) * support_mask[:,None]  [B,C]
    mask_support = sum(y_rule, -1)                     [B]
    y_ml = x @ Wml + bml                               [B,C]
    mix  = g0*y_ml + g1*y_rule        (row 0)
         = g0*(y_ml + g1*y_rule)      (rows > 0)
    y    = where(mask_support != 0, mix, y_ml)
Returns (y, gate, mask_support).

Strategy: pure data parallel over 8 NeuronCores (2048 rows each, weights
replicated). The two D*C matmuls (99.8% of FLOPs) run on the PE at fp32r
full rate (N=512 moving); x is pre-transposed host-side so the contraction
dim D sits on partitions without burning PE time on transposes, and the
tiny gating network (x @ Wg is 0.1% of FLOPs) plus the per-row softmax
coefficients are computed host-side in fp32 — measured on HW, N=2 matmuls
and ACT-table switches cost more than the whole device epilogue.

Per 128-row tile on device:
  - 8 accumulating fp32r matmuls per output half into PSUM for each expert,
  - relu on ScalarE with per-row scale b*mask folded in (relu(b*m*z) ==
    b*m*relu(z) for b*m >= 0) and accum_out producing the scaled row sum,
  - y = (psum_ml * a) + y_rule_scaled as one fused DVE op per half.
Per-row coefficients (host): b = g1 on global row 0 else g0*g1; device
derives a = g0 if any rule fired (scaled row sum != 0) else 1, and
mask_support = row_sum / b.
"""

import numpy as np

B, D, C = 16384, 1024, 1024
N_CORES = 8
BS = B // N_CORES          # 2048 rows per core
P = 128                    # partitions
TB = BS // P               # 16 row-tiles per core
KC = D // P                # 8 contraction chunks
NT = 512                   # moving free dim per matmul
CT = C // NT               # 2 column tiles

_BUILD_CACHE = {}


def _build(has_bml, has_br, repeat=1):
    import concourse.tile as tile
    from concourse import bacc, mybir
    from concourse.bass import ts

    f32 = mybir.dt.float32
    f32r = mybir.dt.float32r
    Alu = mybir.AluOpType
    Act = mybir.ActivationFunctionType

    nc = bacc.Bacc("TRN2", target_bir_lowering=False, debug=False)

    # x pre-transposed host-side: [p, t, k, bb] = x[t*P + bb, k*P + p],
    # so one row-tile's x^T is a single contiguous 4 KB run per partition
    xt_d = nc.dram_tensor("xt", [P, TB, KC, P], f32r, kind="ExternalInput").ap()
    wml_d = nc.dram_tensor("wml", [D, C], f32r, kind="ExternalInput").ap()
    wr_d = nc.dram_tensor("wr", [D, C], f32r, kind="ExternalInput").ap()
    # host-computed per-row coefficients, [p, t] = row t*P + p
    g0_d = nc.dram_tensor("g0v", [P, TB], f32, kind="ExternalInput").ap()
    u_d = nc.dram_tensor("uv", [P, TB], f32, kind="ExternalInput").ap()     # 1-g0
    brl_d = nc.dram_tensor("brlv", [P, TB], f32, kind="ExternalInput").ap() # b*mask
    binv_d = nc.dram_tensor("binvv", [P, TB], f32, kind="ExternalInput").ap()  # 1/b
    bias_d = {}
    if has_bml:
        bias_d["bml"] = nc.dram_tensor("bml", [C], f32r, kind="ExternalInput").ap()
    if has_br:
        bias_d["br"] = nc.dram_tensor("br", [C], f32r, kind="ExternalInput").ap()

    y_d = nc.dram_tensor("y", [BS, C], f32, kind="ExternalOutput").ap()
    ms_d = nc.dram_tensor("ms", [P, TB], f32, kind="ExternalOutput").ap()

    with tile.TileContext(nc) as tc:
        w_bufs = 2 if repeat > 1 else 1   # double-buffer streams across reps
        with (
            tc.tile_pool(name="wpool", bufs=1) as wpool,
            tc.tile_pool(name="wstream", bufs=w_bufs) as wspool,
            tc.tile_pool(name="xtpool", bufs=3) as xtpool,
            tc.tile_pool(name="cpool", bufs=1) as cpool,
            tc.tile_pool(name="yrpool", bufs=4) as yrpool,
            tc.tile_pool(name="ypool", bufs=4) as ypool,
            tc.tile_pool(name="scpool", bufs=3) as scpool,
            tc.tile_pool(name="pacc", bufs=8, space="PSUM") as pacc_pool,
        ):
            bias_sb = {}
            if bias_d:
                onesf = cpool.tile([1, P], f32, tag="onesf")
                nc.vector.memset(onesf[:], 1.0)
                ones_t = cpool.tile([1, P], f32r, tag="ones")
                nc.vector.tensor_copy(ones_t[:], onesf[:])
                for key, ap in bias_d.items():
                    bt = cpool.tile([1, ap.shape[0]], f32r, tag=f"b_{key}",
                                    name=f"b_{key}")
                    nc.sync.dma_start(bt[:], ap.rearrange("(o c) -> o c", o=1))
                    bias_sb[key] = bt

            coef_sb = {}
            for key, ap in [("g0", g0_d), ("u", u_d), ("brl", brl_d),
                            ("binv", binv_d)]:
                ct_ = wpool.tile([P, TB], f32, tag=f"c_{key}", name=f"c_{key}")
                nc.sync.dma_start(ct_[:], ap)
                coef_sb[key] = ct_

            ms_sb = wpool.tile([P, TB], f32, tag="ms_acc")

            wml_r = wml_d.rearrange("(k p) c -> p k c", p=P)
            wr_r = wr_d.rearrange("(k p) c -> p k c", p=P)
            for rep in range(repeat):
                # weight streams sit inside the repeat loop (double-buffered
                # for repeat>1 timing builds); fine-grained 256 KB chunks in
                # k order, rule weights first — tiles consume them first
                wml_sb = wspool.tile([P, KC, C], f32r, tag="wml")
                wr_sb = wspool.tile([P, KC, C], f32r, tag="wr")
                for k in range(KC):
                    for h in range(2):
                        nc.sync.dma_start(
                            wr_sb[:, k, ts(h, NT)], wr_r[:, k, ts(h, NT)]
                        )
                        nc.sync.dma_start(
                            wml_sb[:, k, ts(h, NT)], wml_r[:, k, ts(h, NT)]
                        )

                for t in range(TB):
                    xt_t = xtpool.tile([P, KC, P], f32r, tag="xt")
                    nc.sync.dma_start(xt_t[:], xt_d[:, t])
                    # ---- rule expert first: its relu -> acf chain hides
                    # under the ml matmuls that follow ----
                    ps_r = [
                        pacc_pool.tile([P, NT], f32, tag="acc", name=f"ps_r{c}")
                        for c in range(CT)
                    ]
                    for k in range(KC):
                        lhs = xt_t[:, k, :]
                        for c in range(CT):
                            nc.tensor.matmul(
                                ps_r[c][:], lhs, wr_sb[:, k, ts(c, NT)],
                                start=(k == 0), stop=(k == KC - 1) and not has_br,
                            )
                    if has_br:
                        for c in range(CT):
                            nc.tensor.matmul(
                                ps_r[c][:], ones_t[:],
                                bias_sb["br"][:, ts(c, NT)],
                                start=False, stop=True,
                            )

                    # scaled relu + row-sum on ScalarE (frees the r banks)
                    brl = coef_sb["brl"][:, t : t + 1]
                    yr = []
                    rs = []
                    for c in range(CT):
                        yr_c = yrpool.tile([P, NT], f32, tag="yr", name=f"yr{c}")
                        rs_c = scpool.tile([P, 1], f32, tag=f"rs{c}", name=f"rs{c}")
                        nc.scalar.activation(
                            yr_c[:], ps_r[c][:], Act.Relu,
                            scale=brl, accum_out=rs_c[:],
                        )
                        yr.append(yr_c)
                        rs.append(rs_c)
                    rsum = scpool.tile([P, 1], f32, tag="rsum")
                    nc.vector.tensor_add(rsum[:], rs[0][:], rs[1][:])

                    # supported <=> rsum != 0 ; a = g0 + (1-g0)*[rsum == 0]
                    wu = scpool.tile([P, 1], f32, tag="wu")
                    nc.vector.tensor_scalar(wu[:], rsum[:], 0.0, None, Alu.is_equal)
                    au = scpool.tile([P, 1], f32, tag="au")
                    nc.vector.tensor_mul(au[:], wu[:], coef_sb["u"][:, t : t + 1])
                    acf = scpool.tile([P, 1], f32, tag="acf")
                    nc.vector.tensor_add(acf[:], au[:], coef_sb["g0"][:, t : t + 1])

                    # mask_support = rsum / b  (0 for unsupported rows)
                    nc.vector.tensor_mul(
                        ms_sb[:, t : t + 1], rsum[:], coef_sb["binv"][:, t : t + 1]
                    )

                    # ---- ml expert; acf is ready by the time it finishes ----
                    ps_ml = [
                        pacc_pool.tile([P, NT], f32, tag="acc", name=f"ps_ml{c}")
                        for c in range(CT)
                    ]
                    for k in range(KC):
                        lhs = xt_t[:, k, :]
                        for c in range(CT):
                            nc.tensor.matmul(
                                ps_ml[c][:], lhs, wml_sb[:, k, ts(c, NT)],
                                start=(k == 0), stop=(k == KC - 1) and not has_bml,
                            )
                    if has_bml:
                        for c in range(CT):
                            nc.tensor.matmul(
                                ps_ml[c][:], ones_t[:],
                                bias_sb["bml"][:, ts(c, NT)],
                                start=False, stop=True,
                            )

                    # final mix: y = a*psum_ml + yr (frees the ml banks)
                    for c in range(CT):
                        y_sb = ypool.tile([P, NT], f32, tag="y", name=f"y{c}")
                        nc.vector.scalar_tensor_tensor(
                            y_sb[:], ps_ml[c][:], acf[:, 0:1], yr[c][:],
                            Alu.mult, Alu.add,
                        )
                        nc.sync.dma_start(y_d[ts(t, P), ts(c, NT)], y_sb[:])

            nc.sync.dma_start(ms_d[:], ms_sb[:])

    nc.compile()
    return nc


def _prepare(x, Wg, bg, Wml, bml, Wr, br, support_mask):
    """Host-side prep: flags, per-core in_maps, and the gate output."""
    x = np.ascontiguousarray(np.asarray(x, dtype=np.float32))
    Wg = np.asarray(Wg, dtype=np.float32)
    Wml = np.ascontiguousarray(np.asarray(Wml, dtype=np.float32))
    Wr = np.ascontiguousarray(np.asarray(Wr, dtype=np.float32))
    bg = np.asarray(bg, dtype=np.float32)
    bml = np.asarray(bml, dtype=np.float32)
    br = np.asarray(br, dtype=np.float32)
    support_mask = np.asarray(support_mask)

    flags = (bool(bml.any()), bool(br.any()))

    # ---- host-side gating network (0.1% of FLOPs) + per-row coefficients ----
    logits = x @ Wg + bg                       # [B, 2] fp32
    m = logits.max(axis=1, keepdims=True)
    e = np.exp(logits - m)
    gate = (e / e.sum(axis=1, keepdims=True)).astype(np.float32)
    g0 = np.ascontiguousarray(gate[:, 0])
    g1 = np.ascontiguousarray(gate[:, 1])
    u = (np.float32(1.0) - g0).astype(np.float32)
    ssel = g0.copy()
    ssel[0] = np.float32(1.0)                  # row-0 asymmetry: b = g1 there
    b_coef = (g1 * ssel).astype(np.float32)
    mask_f = support_mask.astype(np.float32)
    brl = (b_coef * mask_f).astype(np.float32)
    binv = (np.float32(1.0) / b_coef).astype(np.float32)

    def per_core_cols(v, sl):                  # [BS] -> [P, TB]
        return np.ascontiguousarray(v[sl].reshape(TB, P).T)

    in_maps = []
    for cix in range(N_CORES):
        sl = slice(cix * BS, (cix + 1) * BS)
        xs = x[sl]
        # [p, t, k, bb] = xs[t*P + bb, k*P + p]
        xt = np.ascontiguousarray(
            xs.T.reshape(KC, P, TB, P).transpose(1, 2, 0, 3)
        )
        m_ = {
            "xt": xt,
            "wml": Wml,
            "wr": Wr,
            "g0v": per_core_cols(g0, sl),
            "uv": per_core_cols(u, sl),
            "brlv": per_core_cols(brl, sl),
            "binvv": per_core_cols(binv, sl),
        }
        if flags[0]:
            m_["bml"] = bml
        if flags[1]:
            m_["br"] = br
        in_maps.append(m_)
    return flags, in_maps, gate


def kernel(x, Wg, bg, Wml, bml, Wr, br, support_mask):
    from concourse.bass_utils import run_bass_kernel_spmd

    flags, in_maps, gate = _prepare(x, Wg, bg, Wml, bml, Wr, br, support_mask)
    if flags not in _BUILD_CACHE:
        _BUILD_CACHE[flags] = _build(*flags)
    nc = _BUILD_CACHE[flags]

    res = run_bass_kernel_spmd(nc, in_maps, core_ids=list(range(N_CORES)))

    y = np.concatenate([res.results[c]["y"] for c in range(N_CORES)], axis=0)
    ms = np.concatenate(
        [res.results[c]["ms"].T.reshape(BS) for c in range(N_CORES)], axis=0
    )
    return y, gate, ms
